# revision 6
# baseline (speedup 1.0000x reference)
"""Trainium2 Bass kernel for nn_BlockAttnRes.

Reference computation (B=4, N=8, S=4096, D=1024):
    partial   = partial_block + current                      [B,S,D]
    summaries = rmsnorm(block_outputs[:, :, -1, :]) * rms_w  [B,N,D]
    query     = partial[:, -1, :] @ res_proj_w.T             [B,D]
    scores    = einsum("bd,bnd->bn", query, summaries)/sqrt(D)
    weights   = softmax(scores, axis=-1)                     [B,N]
    attended  = einsum("bn,bnsd->bsd", weights, block_outputs)
    returns (partial + attended, partial)

Sharding: 8 cores, core c -> (b = c//2, s-half = c%2). Each core gets its
batch's S/2 slice of current/partial_block/block_outputs plus the (tiny)
last-token slices + replicated weights, computes its own softmax weights
(no cross-core communication), and produces its S/2 slice of both outputs.

The big work is the weighted sum over N=8 block_outputs: pure DMA-bound
streaming (64 MiB of block_outputs per core) with a DVE multiply-accumulate
chain using per-partition scalar operands.
"""

from contextlib import ExitStack

import numpy as np

import concourse.bacc as bacc
import concourse.bass as bass
import concourse.mybir as mybir
import concourse.tile as tile
from concourse import masks
from concourse.bass_utils import run_bass_kernel_spmd

F32 = mybir.dt.float32
FP32_EPS = float(np.finfo(np.float32).eps)

B, N, S, D = 4, 8, 4096, 1024
NCORES = 8
S_SH = S // 2               # 2048 sequence rows per core
P = 128                     # SBUF partitions
TWO = 2                     # s-rows packed per partition (contiguous in DRAM)
FREE = TWO * D              # 2048 f32 = 8KB per partition row -> 1MiB tiles
NT = S_SH // (P * TWO)      # 8 tiles per core
INV_SQRT_D = 1.0 / 32.0     # 1/sqrt(1024)


def _build_score_path(nc, tc, ctx, small, psum, bol, curl, pbl, w, rw):
    """Emit the tiny per-core softmax-weight computation.

    Returns wb: SBUF tile [P, N] with weights[n] broadcast to all partitions.
    """
    mult, add = mybir.AluOpType.mult, mybir.AluOpType.add

    # --- summaries = rmsnorm(bo_last) * rms_w : [N, D] ---
    summ = small.tile([N, D], F32)
    nc.sync.dma_start(out=summ[:], in_=bol.ap())
    x2 = small.tile([N, D], F32)
    nc.vector.tensor_mul(out=x2[:], in0=summ[:], in1=summ[:])
    nsub = D // nc.vector.BN_STATS_FMAX  # 2 subgroups of 512
    stats = small.tile([N, nsub, nc.vector.BN_STATS_DIM], F32)
    x2r = x2[:].rearrange("p (s f) -> p s f", s=nsub)
    for i in range(nsub):
        nc.vector.bn_stats(out=stats[:, i, :], in_=x2r[:, i, :])
    mv = small.tile([N, nc.vector.BN_AGGR_DIM], F32)
    nc.vector.bn_aggr(out=mv[:], in_=stats[:])
    # rstd = 1/sqrt(mean(x^2) + eps)
    eps_t = small.tile([N, 1], F32)
    nc.vector.memset(eps_t[:], FP32_EPS)
    rstd = small.tile([N, 1], F32)
    nc.scalar.activation(
        out=rstd[:], in_=mv[:, 0:1],
        func=mybir.ActivationFunctionType.Sqrt, bias=eps_t[:], scale=1.0,
    )
    nc.vector.reciprocal(out=rstd[:], in_=rstd[:])
    nc.vector.tensor_scalar_mul(out=summ[:], in0=summ[:], scalar1=rstd[:])
    # * rms_w (broadcast along partitions via 0-stride DMA)
    rwb = small.tile([N, D], F32)
    rw_ap = rw.ap()
    nc.gpsimd.dma_start(
        out=rwb[:],
        in_=bass.AP(tensor=rw_ap.tensor, offset=rw_ap.offset,
                    ap=[[0, N], list(rw_ap.ap[-1])]),
    )
    nc.vector.tensor_mul(out=summ[:], in0=summ[:], in1=rwb[:])

    # --- pl = (partial_block + current) last token : [1, D] ---
    cl = small.tile([1, D], F32)
    nc.sync.dma_start(out=cl[:], in_=curl.ap())
    pbt = small.tile([1, D], F32)
    nc.sync.dma_start(out=pbt[:], in_=pbl.ap())
    pl = small.tile([1, D], F32)
    nc.vector.tensor_add(out=pl[:], in0=cl[:], in1=pbt[:])

    # --- transposes (PE): sT[do, n] and plT[di, 1] per 128-chunk ---
    ident = small.tile([P, P], F32)
    masks.make_identity(nc, ident[:])
    KC = D // P  # 8 chunks
    sT = small.tile([P, KC, N], F32)
    plT = small.tile([P, KC], F32)
    for k in range(KC):
        ps_s = psum.tile([P, N], F32, tag="trs", bufs=1)
        nc.tensor.transpose(ps_s[:], summ[:, k * P:(k + 1) * P], ident[:N, :N])
        nc.vector.tensor_copy(out=sT[:, k, :], in_=ps_s[:])
        ps_p = psum.tile([P, 1], F32, tag="trp", bufs=1)
        nc.tensor.transpose(ps_p[:], pl[:, k * P:(k + 1) * P], ident[:1, :1])
        nc.vector.tensor_copy(out=plT[:, k:k + 1], in_=ps_p[:])

    # --- u^T[di, n] = sum_do W[do, di] * s[n, do], streamed over do-chunks.
    # NOTE: matmul start=True zeroes the WHOLE 2KB PSUM bank, so every
    # matmul gets its own single-shot group; accumulation over do-chunks
    # happens in SBUF via DVE (tiny [128,8] adds).
    w_ap = w.ap()
    u_sb = small.tile([P, KC, N], F32)
    wpool = ctx.enter_context(tc.tile_pool(name="wpool", bufs=3))
    for j in range(KC):
        wj = wpool.tile([P, D], F32, tag="wj")
        nc.sync.dma_start(out=wj[:], in_=w_ap[j * P:(j + 1) * P, :])
        for k in range(KC):
            mp = psum.tile([P, N], F32, tag="mm", bufs=4)
            nc.tensor.matmul(
                mp[:],
                lhsT=wj[:, k * P:(k + 1) * P],
                rhs=sT[:, j, :],
                start=True, stop=True,
            )
            if j == 0:
                nc.vector.tensor_copy(out=u_sb[:, k, :], in_=mp[:])
            else:
                nc.vector.tensor_add(out=u_sb[:, k, :],
                                     in0=u_sb[:, k, :], in1=mp[:])

    # --- scores[n] = sum_di pl[di] * u^T[di, n], then softmax ---
    sc_ps = psum.tile([1, N], F32, tag="scps", bufs=1)
    for k in range(KC):
        nc.tensor.matmul(
            sc_ps[:], lhsT=plT[:, k:k + 1], rhs=u_sb[:, k, :],
            start=(k == 0), stop=(k == KC - 1),
        )
    sc = small.tile([1, N], F32)
    nc.scalar.mul(sc[:], sc_ps[:], INV_SQRT_D)
    mx = small.tile([1, 1], F32)
    nc.vector.reduce_max(out=mx[:], in_=sc[:], axis=mybir.AxisListType.X,
                         negate=True)
    ex = small.tile([1, N], F32)
    nc.scalar.activation(out=ex[:], in_=sc[:],
                         func=mybir.ActivationFunctionType.Exp,
                         bias=mx[:], scale=1.0)
    sm = small.tile([1, 1], F32)
    nc.vector.reduce_sum(out=sm[:], in_=ex[:], axis=mybir.AxisListType.X)
    rcp = small.tile([1, 1], F32)
    nc.vector.reciprocal(rcp[:], sm[:])
    wsm = small.tile([1, N], F32)
    nc.vector.tensor_scalar_mul(out=wsm[:], in0=ex[:], scalar1=rcp[:])

    # --- broadcast weights to all 128 partitions via ones-matmul ---
    ones = small.tile([1, P], F32)
    nc.vector.memset(ones[:], 1.0)
    wb_ps = psum.tile([P, N], F32, tag="wbps", bufs=1)
    nc.tensor.matmul(wb_ps[:], lhsT=ones[:], rhs=wsm[:], start=True, stop=True)
    wb = small.tile([P, N], F32)
    nc.vector.tensor_copy(out=wb[:], in_=wb_ps[:])
    return wb


def _build():
    mult, add = mybir.AluOpType.mult, mybir.AluOpType.add
    nc = bacc.Bacc("TRN2", target_bir_lowering=False, debug=False)

    bo = nc.dram_tensor("bo", [N, S_SH, D], F32, kind="ExternalInput")
    cur = nc.dram_tensor("cur", [S_SH, D], F32, kind="ExternalInput")
    pb = nc.dram_tensor("pb", [S_SH, D], F32, kind="ExternalInput")
    bol = nc.dram_tensor("bol", [N, D], F32, kind="ExternalInput")
    curl = nc.dram_tensor("curl", [1, D], F32, kind="ExternalInput")
    pbl = nc.dram_tensor("pbl", [1, D], F32, kind="ExternalInput")
    w = nc.dram_tensor("w", [D, D], F32, kind="ExternalInput")
    rw = nc.dram_tensor("rw", [1, D], F32, kind="ExternalInput")
    out0 = nc.dram_tensor("out0", [S_SH, D], F32, kind="ExternalOutput")
    out1 = nc.dram_tensor("out1", [S_SH, D], F32, kind="ExternalOutput")

    with tile.TileContext(nc) as tc, ExitStack() as ctx:
        small = ctx.enter_context(tc.tile_pool(name="small", bufs=1))
        psum = ctx.enter_context(tc.tile_pool(name="psum", bufs=1, space="PSUM"))

        with ExitStack() as pctx:
            wb = _build_score_path(nc, tc, pctx, small, psum,
                                   bol, curl, pbl, w, rw)

        # ---- main loop: stream 1MiB tiles, DVE weighted-accumulate ----
        bo_r = bo.ap().rearrange("n (t p two) d -> n t p (two d)", p=P, two=TWO)
        cur_r = cur.ap().rearrange("(t p two) d -> t p (two d)", p=P, two=TWO)
        pb_r = pb.ap().rearrange("(t p two) d -> t p (two d)", p=P, two=TWO)
        o0_r = out0.ap().rearrange("(t p two) d -> t p (two d)", p=P, two=TWO)
        o1_r = out1.ap().rearrange("(t p two) d -> t p (two d)", p=P, two=TWO)

        with tc.tile_pool(name="bop", bufs=12) as bop, \
             tc.tile_pool(name="iop", bufs=2) as iop:
            for t in range(NT):
                ct = iop.tile([P, FREE], F32, tag="ct")
                nc.sync.dma_start(out=ct[:], in_=cur_r[t])
                pt = iop.tile([P, FREE], F32, tag="pt")
                nc.sync.dma_start(out=pt[:], in_=pb_r[t])
                bts = []
                for n in range(N):
                    bt = bop.tile([P, FREE], F32, tag="bt")
                    nc.sync.dma_start(out=bt[:], in_=bo_r[n, t])
                    bts.append(bt)
                # partial = current + partial_block (in place in ct)
                nc.vector.tensor_add(out=ct[:], in0=ct[:], in1=pt[:])
                nc.scalar.dma_start(out=o1_r[t], in_=ct[:])
                # acc = sum_n w[n] * bo[n]
                acc = iop.tile([P, FREE], F32, tag="acc")
                nc.vector.tensor_scalar_mul(out=acc[:], in0=bts[0][:],
                                            scalar1=wb[:, 0:1])
                for n in range(1, N):
                    nc.vector.scalar_tensor_tensor(
                        out=acc[:], in0=bts[n][:], scalar=wb[:, n:n + 1],
                        in1=acc[:], op0=mult, op1=add,
                    )
                nc.vector.tensor_add(out=acc[:], in0=acc[:], in1=ct[:])
                nc.scalar.dma_start(out=o0_r[t], in_=acc[:])

    nc.compile()
    return nc


def _run(in_maps, trace=False):
    nc = _build()
    return run_bass_kernel_spmd(nc, in_maps, core_ids=list(range(NCORES)),
                                trace=trace)


def _make_in_maps(current, block_outputs, partial_block, res_proj_w, rms_w):
    current = np.asarray(current, dtype=np.float32)
    block_outputs = np.asarray(block_outputs, dtype=np.float32)
    partial_block = np.asarray(partial_block, dtype=np.float32)
    res_proj_w = np.ascontiguousarray(np.asarray(res_proj_w, dtype=np.float32))
    rms_w = np.asarray(rms_w, dtype=np.float32).reshape(1, D)
    in_maps = []
    for c in range(NCORES):
        b, h = divmod(c, 2)
        s0 = h * S_SH
        in_maps.append({
            "bo": np.ascontiguousarray(block_outputs[b, :, s0:s0 + S_SH, :]),
            "cur": np.ascontiguousarray(current[b, s0:s0 + S_SH, :]),
            "pb": np.ascontiguousarray(partial_block[b, s0:s0 + S_SH, :]),
            "bol": np.ascontiguousarray(block_outputs[b, :, -1, :]),
            "curl": np.ascontiguousarray(current[b, -1:, :]),
            "pbl": np.ascontiguousarray(partial_block[b, -1:, :]),
            "w": res_proj_w,
            "rw": np.ascontiguousarray(rms_w),
        })
    return in_maps


def _gather(results):
    out0 = np.empty((B, S, D), np.float32)
    out1 = np.empty((B, S, D), np.float32)
    for c in range(NCORES):
        b, h = divmod(c, 2)
        s0 = h * S_SH
        out0[b, s0:s0 + S_SH, :] = results[c]["out0"]
        out1[b, s0:s0 + S_SH, :] = results[c]["out1"]
    return out0, out1


def kernel(current, block_outputs, partial_block, res_proj_w, rms_w):
    in_maps = _make_in_maps(current, block_outputs, partial_block,
                            res_proj_w, rms_w)
    res = _run(in_maps, trace=False)
    return _gather(res.results)
